# revision 7
# baseline (speedup 1.0000x reference)
"""Bahdanau (additive) attention TRN2 Bass kernel — v2, S-on-partitions.

reference:
    proj_in = einsum("bse,ea->bsa", inputs, W_in)      # [B,S,A]
    proj_q  = (query @ W_q)[:, None, :]                # [B,1,A]
    scores  = einsum("bsa,a->bs", tanh(proj_in+proj_q), w_att)
    weights = softmax(scores, axis=1)
    context = einsum("bs,bsa->ba", weights, proj_in)   # [B,A]

B,S,E,Q,A = 32,2048,1024,1024,512.

Sharding: data-parallel over batch. 8 cores x 4 batches; weights
replicated. proj_q is precomputed on the host (0.05% of FLOPs) and
shipped pre-broadcast across partitions, as is w_att.

Device algorithm (per batch; main matmul produces proj_in[s_tile,a]
with the SEQUENCE dim on partitions, unlike v1's [a,s] orientation):
  - per s_tile (16 of them): 8 e-chunk matmuls accumulate
    psum[s=128, a=512]; DVE drains psum once, fusing the +proj_q bias
    (partition-broadcast tile) and the bf16 cast -> pb.
  - ACT tanh(pb) -> t; one DVE scalar_tensor_tensor computes
    (t * w_att_bcast) with accum_out = scores[:,st] (fused mult+reduce
    over the free dim -- softmax scores land directly ON partitions).
  - ACT exp(scores[:,st]) -> expbf[:,st] (bf16, no max-subtraction:
    |scores| <= ~3). No cross-partition broadcast is ever needed.
  - context becomes PE matmuls: ctx[1,a] += expbf[:,st].T @ pb(st),
    emitted CTX_LAG s_tiles behind the main stream so the PE never
    head-blocks on the DVE/ACT chain. Denominator = ones.T @ expbf
    (one matmul) + tiny reduce/reciprocal.
  - out_row = ctx * (1/total) - projq_bf16: since pb = proj_in +
    bf16(projq), subtracting the SAME bf16 projq cancels the bias
    exactly; normalization error only multiplies the residual.
  - batch b's last CTX_LAG context matmuls + finalize interleave into
    batch b+1's main stream; only batch 3's ~2.5us chain is a tail.
  - batch 0's X tiles load in s-quarter chunks so the first matmuls
    start ~3us in instead of waiting for the full 4.2MB stripe set.
"""

import sys

sys.path.insert(0, "/opt/trn_rl_repo")

import ml_dtypes
import numpy as np

import concourse.bass as bass
import concourse.tile as tile
from concourse import bacc, bass_utils, mybir

B, S, E, Q, A = 32, 2048, 1024, 1024, 512
NCORES = 8
BPC = B // NCORES  # batches per core
P = 128
EC = E // P  # 8 e-chunks (contraction)
ST = S // P  # 16 s-tiles per batch (output partition tiles)
NQ = 4  # batch-0 load quarters
QW = S // NQ
CTX_LAG = 3  # context matmul trails the main stream by this many s_tiles

BF = mybir.dt.bfloat16
F32 = mybir.dt.float32
TANH = mybir.ActivationFunctionType.Tanh
EXP = mybir.ActivationFunctionType.Exp


def build():
    nc = bacc.Bacc("TRN2", target_bir_lowering=False, debug=False)

    xT = nc.dram_tensor("xT", [BPC, E, S], BF, kind="ExternalInput")
    w_in = nc.dram_tensor("w_in", [E, A], BF, kind="ExternalInput")
    pqbc = nc.dram_tensor("pqbc", [P, BPC * A], BF, kind="ExternalInput")
    wabc = nc.dram_tensor("wabc", [P, A], BF, kind="ExternalInput")
    out = nc.dram_tensor("out", [BPC, A], F32, kind="ExternalOutput")

    with tile.TileContext(nc) as tc:
        with (
            tc.tile_pool(name="const", bufs=1) as const,
            tc.tile_pool(name="xtp", bufs=2) as xtp,
            tc.tile_pool(name="pbp", bufs=8) as pbp,
            tc.tile_pool(name="tp", bufs=3) as tp,
            tc.tile_pool(name="small", bufs=2) as small,
            tc.tile_pool(name="mm_ps", bufs=5, space="PSUM") as mm_ps,
            tc.tile_pool(name="ctx_ps", bufs=2, space="PSUM") as ctx_ps,
            tc.tile_pool(name="sum_ps", bufs=1, space="PSUM") as sum_ps,
        ):
            w_sb = const.tile([P, EC, A], BF)
            w_r = w_in.ap().rearrange("(ec p) a -> p ec a", p=P)
            wabc_sb = const.tile([P, A], BF)
            pqbc_sb = const.tile([P, BPC * A], BF)
            ones_sb = const.tile([P, 1], BF)
            nc.vector.memset(ones_sb, 1.0)
            # weights + broadcast constants dispatch on the scalar/gpsimd
            # DMA rings so the sync ring only streams X (one ~0.6us
            # dispatch each; a single dma_start fans out across all 16
            # SDMA engines, so fewer+bigger beats many small ones)
            nc.scalar.dma_start(w_sb, w_r)
            nc.gpsimd.dma_start(wabc_sb, wabc.ap())
            nc.gpsimd.dma_start(pqbc_sb, pqbc.ap())

            state = {}  # batch -> dict(pbs, expbf, scores, cp)

            def emit_ctx(b, st):
                st_ = state[b]
                nc.tensor.matmul(
                    st_["cp"],
                    st_["expbf"][:, st : st + 1],
                    st_["pbs"][st],
                    start=(st == 0),
                    stop=(st == ST - 1),
                )

            def finalize(b):
                st_ = state[b]
                sp = sum_ps.tile([1, ST], F32, name="sum")
                nc.tensor.matmul(sp, ones_sb, st_["expbf"], start=True, stop=True)
                tot = small.tile([1, 1], F32, name="tot")
                nc.vector.tensor_reduce(
                    tot, sp, axis=mybir.AxisListType.X, op=mybir.AluOpType.add
                )
                rcp = small.tile([1, 1], F32, name="rcp")
                nc.vector.reciprocal(rcp, tot)
                orow = small.tile([1, A], F32, name="orow")
                # orow = (ctx * 1/total) - bf16(projq), one fused DVE op
                nc.vector.scalar_tensor_tensor(
                    out=orow,
                    in0=st_["cp"],
                    scalar=rcp,
                    in1=pqbc_sb[0:1, b * A : (b + 1) * A],
                    op0=mybir.AluOpType.mult,
                    op1=mybir.AluOpType.subtract,
                )
                nc.sync.dma_start(out.ap()[b : b + 1, :], orow)
                del state[b]

            for b in range(BPC):
                xt_all = xtp.tile([P, EC, S], BF, name="xt")
                xr = xT.ap()[b].rearrange("(ec p) s -> p ec s", p=P)
                if b == 0:
                    # per-ec stripe loads: 4KB contiguous runs keep the
                    # descriptor count (and ~5ns/descriptor HWDGE dispatch
                    # cost) low, and the first matmuls start on stripe 0
                    for ec in range(EC):
                        nc.sync.dma_start(xt_all[:, ec, :], xr[:, ec, :])
                else:
                    nc.sync.dma_start(xt_all, xr)

                scores = small.tile([P, ST], F32, name="scores")
                expbf = small.tile([P, ST], BF, name="expbf")
                cp = ctx_ps.tile([1, A], F32, name="ctx")
                state[b] = {"pbs": [], "expbf": expbf, "scores": scores, "cp": cp}

                for st in range(ST):
                    ps = mm_ps.tile([P, A], F32, name="mm")
                    for ec in range(EC):
                        nc.tensor.matmul(
                            ps,
                            xt_all[:, ec, st * P : (st + 1) * P],
                            w_sb[:, ec, :],
                            start=(ec == 0),
                            stop=(ec == EC - 1),
                        )
                    # single PSUM reader: drain + bias + bf16 cast in one op
                    pb = pbp.tile([P, A], BF, name="pb")
                    nc.vector.tensor_tensor(
                        out=pb,
                        in0=ps,
                        in1=pqbc_sb[:, b * A : (b + 1) * A],
                        op=mybir.AluOpType.add,
                    )
                    state[b]["pbs"].append(pb)
                    t = tp.tile([P, A], BF, name="t")
                    nc.scalar.activation(t, pb, TANH)
                    # fused (t * w_att) with free-dim accumulation -> scores
                    scr = tp.tile([P, A], BF, name="scr", bufs=2)
                    nc.vector.scalar_tensor_tensor(
                        out=scr,
                        in0=t,
                        scalar=0.0,
                        in1=wabc_sb,
                        op0=mybir.AluOpType.bypass,
                        op1=mybir.AluOpType.mult,
                        accum_out=scores[:, st : st + 1],
                    )
                    nc.scalar.activation(
                        expbf[:, st : st + 1], scores[:, st : st + 1], EXP
                    )
                    # trail the main stream with this batch's ctx matmuls
                    if st >= CTX_LAG:
                        emit_ctx(b, st - CTX_LAG)
                    # previous batch's deferred ctx tail + finalize
                    if (b - 1) in state and st < CTX_LAG:
                        emit_ctx(b - 1, ST - CTX_LAG + st)
                        if st == CTX_LAG - 1:
                            finalize(b - 1)

            # last batch's tail
            for st in range(ST - CTX_LAG, ST):
                emit_ctx(BPC - 1, st)
            finalize(BPC - 1)

    nc.compile()
    return nc


def make_in_maps(inputs, query, W_in, W_q, w_att):
    bf = ml_dtypes.bfloat16
    x_bf = np.asarray(inputs).astype(bf)
    xT_bf = np.ascontiguousarray(x_bf.transpose(0, 2, 1))  # [B, E, S]
    w_in_bf = np.ascontiguousarray(np.asarray(W_in).astype(bf))
    projq = np.asarray(query, dtype=np.float32) @ np.asarray(W_q, dtype=np.float32)
    pq_bf = projq.astype(bf)  # [B, A]
    wa_bf = np.asarray(w_att).astype(bf)
    wabc_np = np.ascontiguousarray(np.broadcast_to(wa_bf[None, :], (P, A)))

    in_maps = []
    for c in range(NCORES):
        sl = slice(c * BPC, (c + 1) * BPC)
        pq_row = pq_bf[sl].reshape(1, BPC * A)
        in_maps.append(
            {
                "xT": np.ascontiguousarray(xT_bf[sl]),
                "w_in": w_in_bf,
                "pqbc": np.ascontiguousarray(np.broadcast_to(pq_row, (P, BPC * A))),
                "wabc": wabc_np,
            }
        )
    return in_maps


_nc = None


def kernel(inputs, query, W_in, W_q, w_att):
    global _nc
    if _nc is None:
        _nc = build()

    in_maps = make_in_maps(inputs, query, W_in, W_q, w_att)
    res = bass_utils.run_bass_kernel_spmd(_nc, in_maps, core_ids=list(range(NCORES)))
    return np.concatenate([r["out"] for r in res.results], axis=0)


if __name__ == "__main__":
    rng = np.random.default_rng(0)
    ins = {
        "inputs": rng.standard_normal((B, S, E), dtype=np.float32),
        "query": rng.standard_normal((B, Q), dtype=np.float32),
        "W_in": (rng.standard_normal((E, A), dtype=np.float32) / np.sqrt(E)).astype(
            np.float32
        ),
        "W_q": (rng.standard_normal((Q, A), dtype=np.float32) / np.sqrt(Q)).astype(
            np.float32
        ),
        "w_att": (rng.standard_normal((A,), dtype=np.float32) / np.sqrt(A)).astype(
            np.float32
        ),
    }
    got = kernel(**ins)
    print("out shape", got.shape, got.dtype)
